# revision 37
# baseline (speedup 1.0000x reference)
"""Trainium2 Bass kernel for DifferentiableRBFSVMModel forward.

Math (reference):
    dist[n,s] = max(x_sq[n] + xi_sq[s] - 2*cross[n,s], 0)
    K = exp(-g*dist);  res = sigmoid(K @ (alphas*yis) + intercept)   -> [1, N]

Factorization used on device (clamp dropped: dist >= 0 up to fp eps):
    K[n,s]*w[s] = A[n] * sgn_s * exp(2g*cross[n,s] - g*xi_sq[s] + ln(alpha_s))
    res[n] = sigmoid(A[n] * sum_s sgn_s*E[s,n] + c),  A = exp(-g*x_sq)

SVs are host-sorted so all yis=+1 come first: every 128-row s-tile is then
sign-pure except at most one boundary tile (fixed with a per-partition sgn
multiply).  The weighted SV reduction becomes plain +/- adds of E tiles.

Sharding: data-parallel over N across 8 cores, everything else replicated.
Per core (NS = 2048 rows of x, 64 s-tiles):
    mm1 (PE):  psum tiles [128s x 2048n] = xisT_tile^T @ xT  (fp16, fp32 acc)
    exp:       E_t = exp(2g*psum + bias_s) in fp16.  Most stages on ACT
               ([128,2048] instructions); OFFLOAD stages use a custom DVE op
               EXP8_ANT (8 ALU blocks: (c((1+lam*v)^2+b))^8 * C1, C1 =
               c^8*exp(bias) per partition, max rel err ~3e-4) so the two
               engines split the exp wall.
    DVE:       s-reduction off the PE: per-group-of-8 pairwise fp16 trees
               (last group a chain for a short drain), group roots folded
               into ONE fp32 running accumulator TACC (mixed-dtype adds).
    PE tail:   po[4,512] = pm^T @ TACC (4 fp32 matmuls, selector lhs puts
               n-chunk c in partition c; psum pool opened after the mm1
               pool closes), then z = A*po, sigmoid via tanh on [4,512].

DMA: xisT split into 8 column-chunks per d-half; chunks 1+ gated on a marker
read of the previous stage's E tile so their DMA doesn't compete with the
prologue-critical DMAs.
"""

import numpy as np

N, D, S, NCORES = 16384, 256, 8192, 8
NS = N // NCORES          # 2048 rows of x per core
TS = S // 128             # 64 s-tiles
GAMMA = 0.00390625        # 1/256
XCH = 8                   # xisT column chunks per d-half (1024 cols each)
GRP = 8                   # s-tiles per reduction group

# exp(v) ~= (C_EXP*((1+LAM_EXP*v)^2 + B_EXP))^8, minimax on |v|<=0.8
LAM_EXP = 0.124753797
B_EXP = 0.993573014
C8_EXP = 0.004008243562419464
IMM2_EXP = LAM_EXP * 2.0 * GAMMA          # Src0 is raw cross from psum

# Stages whose exp runs on the DVE (custom op) instead of ACT.  Offloads
# only help in CONSECUTIVE PAIRS: with the 2-deep psum ring, a lone skipped
# ACT stage just stalls ACT on the PE; a pair lets ACT and DVE consume
# alternate psum tiles concurrently and the stream goes PE-bound.
OFFLOAD = frozenset()


def _leaf_signs(p_pos):
    """Per-s-tile sign after host sort; boundary tile is sgn-fixed to +1."""
    signs = []
    boundary = -1
    for t in range(TS):
        lo, hi = t * 128, (t + 1) * 128
        if hi <= p_pos:
            signs.append(1)
        elif lo >= p_pos:
            signs.append(-1)
        else:
            signs.append(1)
            boundary = t
    return signs, boundary


_EXP8 = None


def _get_exp8():
    """Register the EXP8_ANT custom DVE op (idempotent per process)."""
    global _EXP8
    if _EXP8 is not None:
        return _EXP8
    import concourse.dve_ops as dops
    from concourse.dve_ops import DveOp
    from concourse.dve_spec import (
        C0, C1, C2, One, Spec, Src0, _has_src1, lower as uop_lower,
    )
    from concourse.dve_uop import DveOpSpec

    w1 = Src0 * C2 + One
    a2 = w1 * w1 + C0
    s1 = a2 * a2
    s2 = s1 * s1
    s3 = s2 * s2
    body = s3 * C1

    def _ref(in0, in1, s0, s1, imm2):
        a2 = (1.0 + imm2 * in0.astype(np.float32)) ** 2 + s0
        return (a2 ** 8) * s1

    spec = Spec(body=body, reference=_ref)
    if "EXP8_ANT" not in dops._SUB_OPCODE_FOR_NAME:
        row = max(dops._SUB_OPCODE_FOR_NAME.values()) + 1
        assert row < 0x20
        uops = uop_lower(spec, ver="v3")
        sha = DveOpSpec(
            name="EXP8_ANT", opcode=row, uops=uops, rd1_en=_has_src1(spec)
        ).sha("v3")
        op = DveOp("EXP8_ANT", spec, subdim=False, uops_sha={"v3": sha})
        dops.OPS.append(op)
        dops._SUB_OPCODE_FOR_NAME["EXP8_ANT"] = row
        dops.CUSTOM_DVE_SPECS["EXP8_ANT"] = spec
        _EXP8 = op
    else:
        _EXP8 = next(o for o in dops.OPS if o.name == "EXP8_ANT")
    return _EXP8


def _build_bass(p_pos):
    import concourse.bacc as bacc
    import concourse.mybir as mybir
    import concourse.tile as tile

    f32 = mybir.dt.float32
    f16 = mybir.dt.float16
    AF = mybir.ActivationFunctionType
    ALU = mybir.AluOpType

    exp8 = _get_exp8() if OFFLOAD else None
    signs, boundary = _leaf_signs(p_pos)

    nc = bacc.Bacc("TRN2", target_bir_lowering=False, debug=False)

    xT_d = nc.dram_tensor("xT", [2, 128, NS], f16, kind="ExternalInput")
    xisT_d = nc.dram_tensor("xisT", [2, 128, S], f16, kind="ExternalInput")
    biasS_d = nc.dram_tensor("biasS", [128, TS], f32, kind="ExternalInput")
    A4_d = nc.dram_tensor("A4", [4, NS // 4], f32, kind="ExternalInput")
    ch_d = nc.dram_tensor("chalf", [4, 1], f32, kind="ExternalInput")
    sgn_d = nc.dram_tensor("sgn", [128, 1], f32, kind="ExternalInput")
    pm_d = nc.dram_tensor("pm", [128, 16], f32, kind="ExternalInput")
    out_d = nc.dram_tensor("out", [4, NS // 4], f32, kind="ExternalOutput")

    cw = S // XCH  # 1024

    with tile.TileContext(nc) as tc:
        with (
            tc.tile_pool(name="big", bufs=1) as big,
            tc.tile_pool(name="epool", bufs=7) as epool,
            tc.tile_pool(name="ppool", bufs=2) as ppool,
            tc.tile_pool(name="qpool", bufs=2) as qpool,
            tc.tile_pool(name="rpool", bufs=2) as rpool,
            tc.tile_pool(name="c7pool", bufs=2) as c7pool,
            tc.tile_pool(name="tpool", bufs=2) as tpool,
            tc.tile_pool(name="spool", bufs=2) as spool,
            tc.tile_pool(name="e63pool", bufs=4) as e63pool,
        ):
            # Prologue-critical DMAs first, interleaved per d-half; small
            # late-use tensors go through the idle GpSimd queue so they
            # don't delay the Sync issue stream.
            xis = {}
            for c in range(XCH):
                for d in range(2):
                    xis[(d, c)] = big.tile(
                        [128, cw], f16, tag=f"xis{d}_{c}", name=f"xis{d}_{c}"
                    )
            xt = []
            for d in range(2):
                t_ = big.tile([128, NS], f16, tag=f"xt{d}", name=f"xt{d}")
                xt.append(t_)
            biasS = big.tile([128, TS], f32, tag="biasS", name="biasS")
            # Column-ordered prologue: stage 0's LEFT-half exp needs only
            # xisT cols 0-127 (weights) and xt cols 0-1023, so those pieces
            # go first, split across the Sync (d=0) and Vector (d=1) issue
            # queues to halve the serial issue time.
            hN = NS // 2
            nc.sync.dma_start(out=xis[(0, 0)][:, 0:128],
                              in_=xisT_d.ap()[0][:, 0:128])
            nc.gpsimd.dma_start(out=xis[(1, 0)][:, 0:128],
                                in_=xisT_d.ap()[1][:, 0:128])
            nc.sync.dma_start(out=xt[0][:, 0:hN], in_=xT_d.ap()[0][:, 0:hN])
            nc.gpsimd.dma_start(out=xt[1][:, 0:hN], in_=xT_d.ap()[1][:, 0:hN])
            nc.sync.dma_start(out=xt[0][:, hN:NS], in_=xT_d.ap()[0][:, hN:NS])
            nc.gpsimd.dma_start(out=xt[1][:, hN:NS], in_=xT_d.ap()[1][:, hN:NS])
            nc.sync.dma_start(out=xis[(0, 0)][:, 128:cw],
                              in_=xisT_d.ap()[0][:, 128:cw])
            nc.gpsimd.dma_start(out=xis[(1, 0)][:, 128:cw],
                                in_=xisT_d.ap()[1][:, 128:cw])
            nc.gpsimd.dma_start(out=biasS, in_=biasS_d.ap())
            A4 = big.tile([4, NS // 4], f32, tag="A4", name="A4")
            nc.gpsimd.dma_start(out=A4, in_=A4_d.ap())
            chs = big.tile([4, 1], f32, tag="chalf", name="chs")
            nc.gpsimd.dma_start(out=chs, in_=ch_d.ap())
            sgn = big.tile([128, 1], f32, tag="sgn", name="sgn")
            nc.gpsimd.dma_start(out=sgn, in_=sgn_d.ap())
            pm = big.tile([128, 16], f32, tag="pm", name="pm")
            nc.gpsimd.dma_start(out=pm, in_=pm_d.ap())

            # Warmup ACTs: attaches the activation-table-load waits here
            # instead of the first pipeline exp.
            wsrc = big.tile([1, 1], f32, tag="wsrc", name="wsrc")
            nc.vector.memset(wsrc, 0.0)
            wdst = big.tile([1, 1], f32, tag="wdst", name="wdst")
            nc.scalar.activation(wdst, wsrc, AF.Tanh)
            nc.scalar.activation(wdst, wsrc, AF.Exp)

            # PE warmup spin: the PE runs at a low P-state for its first ~3us
            # of activity; burn that on dummy matmuls during the prologue DMA
            # wait so stage 0 streams at full clock.
            wmm = big.tile([128, 512], f16, tag="wmm", name="wmm")
            nc.vector.memset(wmm, 0.0)
            with tc.tile_pool(name="psumw", bufs=1, space="PSUM") as psumw:
                pw = psumw.tile([128, 512], f32, tag="pw", name="pw")
                NWARM = 12
                for i in range(NWARM):
                    nc.tensor.matmul(
                        pw, wmm[:, 0:128], wmm,
                        start=(i == 0), stop=(i == NWARM - 1),
                    )
                nc.vector.tensor_copy(wdst, pw[0:1, 0:1])

            gate = big.tile([1, XCH], f32, tag="gate", name="gate")

            # Reduction with compile-time sign bookkeeping: nodes are
            # (tile, sign); combine(a, b) = a.tile +/- b.tile, sign of a.
            pools = {"P": ppool, "Q": qpool, "R": rpool, "C7": c7pool}

            def comb(a, b, tag, name, pool=None, dtype=f16):
                nt = (pool or pools[tag]).tile([128, NS], dtype, tag=tag, name=name)
                op = ALU.add if a[1] == b[1] else ALU.subtract
                nc.vector.tensor_tensor(nt, a[0], b[0], op)
                return (nt, a[1])

            st = {"tacc": None, "slot_l": None, "slot_p": None,
                  "slot_q": None, "pend_root": None, "chain": None}

            def process_tree(leaf, t):
                """Fold stage t's leaf into the reduction state."""
                g, pos = t // GRP, t % GRP
                root = None
                if g == TS // GRP - 1:
                    # Last group: running fp16 chain -> only one combine
                    # after the final leaf (short drain).
                    st["chain"] = (leaf if pos == 0
                                   else comb(st["chain"], leaf, "C7", f"C7_{t}"))
                elif pos % 2 == 0:
                    st["slot_l"] = leaf
                else:
                    p = comb(st["slot_l"], leaf, "P", f"P_{t}")
                    if pos in (1, 5):
                        st["slot_p"] = p
                    elif pos == 3:
                        st["slot_q"] = comb(st["slot_p"], p, "Q", f"Q_{t}")
                    else:  # pos == 7
                        q2 = comb(st["slot_p"], p, "Q", f"Q2_{t}")
                        root = comb(st["slot_q"], q2, "R", f"R_{t}")
                if root is not None:
                    if st["tacc"] is None and st["pend_root"] is None:
                        st["pend_root"] = root
                    else:
                        a = st["tacc"] if st["tacc"] is not None else st["pend_root"]
                        st["pend_root"] = None
                        st["tacc"] = comb(a, root, "T", f"T_{t}", pool=tpool,
                                          dtype=f32)

            last_e = None
            pend = None   # (leaf, t) tree work delayed one stage so an
            #               offloaded stage's DVE exp leads the DVE queue
            #               and releases its psum tile without queue delay.

            with tc.tile_pool(name="psumc", bufs=2, space="PSUM") as psumc:
                for t in range(TS):
                    c, o = t // XCH, (t % XCH) * 128
                    pc = psumc.tile([128, NS], f32, tag="pc", name=f"pc_{t}")
                    # Stage 0 runs q-major so its left-half psum completes
                    # from the first DMA pieces; later stages run d-major
                    # (2 weight loads per stage instead of 8).
                    dq = ([(d, q) for q in range(4) for d in range(2)]
                          if t == 0 else
                          [(d, q) for d in range(2) for q in range(4)])
                    for d, q in dq:
                        nc.tensor.matmul(
                            pc[:, q * 512 : (q + 1) * 512],
                            xis[(d, c)][:, o : o + 128],
                            xt[d][:, q * 512 : (q + 1) * 512],
                            start=(d == 0),
                            stop=(d == 1),
                        )
                    if t == TS - 1:
                        # Final stage: quarter exps into separate tiles so
                        # each drain quarter depends only on its own exp.
                        e63q = []
                        for qq in range(4):
                            eq = e63pool.tile([128, 512], f16, tag="E63",
                                              name=f"E63_{qq}")
                            nc.scalar.activation(
                                eq, pc[:, qq * 512 : (qq + 1) * 512], AF.Exp,
                                bias=biasS[:, t : t + 1], scale=2.0 * GAMMA,
                            )
                            if t == boundary:
                                nc.vector.tensor_scalar(
                                    out=eq, in0=eq, scalar1=sgn[:, 0:1],
                                    scalar2=None, op0=ALU.mult,
                                )
                            e63q.append(eq)
                        pend = (e63q, signs[t])
                        continue
                    e = epool.tile([128, NS], f16, tag="E", name=f"E_{t}")
                    if t in OFFLOAD:
                        nc.vector._custom_dve(
                            exp8, out=e, in0=pc, s0=B_EXP,
                            s1=expb[:, t : t + 1], imm2=IMM2_EXP,
                        )
                    else:
                        nc.scalar.activation(
                            e, pc, AF.Exp, bias=biasS[:, t : t + 1],
                            scale=2.0 * GAMMA,
                        )
                    if t == boundary:
                        ef = epool.tile([128, NS], f16, tag="E", name=f"Ef_{t}")
                        nc.vector.tensor_scalar(
                            out=ef, in0=e, scalar1=sgn[:, 0:1], scalar2=None,
                            op0=ALU.mult,
                        )
                        e = ef

                    # Gated xisT chunk DMAs: marker waits on the previous
                    # stage's E (SBUF; no extra psum reader), the DMA
                    # WAW-waits on the marker.
                    if t % 4 == 0 and t // 4 + 1 < XCH:
                        cn = t // 4 + 1
                        msrc = last_e if last_e is not None else xt[1]
                        nc.vector.tensor_copy(gate[0:1, cn : cn + 1], msrc[0:1, 0:1])
                        for d in range(2):
                            nc.vector.tensor_copy(
                                xis[(d, cn)][0:1, 0:1], gate[0:1, cn : cn + 1]
                            )
                            nc.sync.dma_start(
                                out=xis[(d, cn)],
                                in_=xisT_d.ap()[d][:, cn * cw : (cn + 1) * cw],
                            )
                    last_e = e

                    process_tree((e, signs[t]), t)

            # Drain, column-split in quarters so the last chain combine, the
            # last fp32 fold and the final matmuls pipeline: the final leaf
            # joins the chain per quarter, folds into TACC per quarter, and
            # each quarter's matmul starts while the next is still on DVE.
            e63q, lsign = pend
            ch = st["chain"]
            opc = ALU.add if ch[1] == lsign else ALU.subtract
            c7f = c7pool.tile([128, NS], f16, tag="C7", name="C7f")
            tacc = st["tacc"]
            opt = ALU.add if tacc[1] == ch[1] else ALU.subtract
            tf = tpool.tile([128, NS], f32, tag="T", name="Tf")

            # Final partition reduction into one [4,512] psum tile at
            # partitions 0-3: 4 fp32 matmuls; pm block cch is +-tacc.sign in
            # column cch and 0 elsewhere, so n-chunk cch lands in row cch.
            with tc.tile_pool(name="psumo", bufs=1, space="PSUM") as psumo:
                po = psumo.tile([4, 512], f32, tag="po", name="po")
                for cch in range(4):
                    sl = slice(cch * 512, (cch + 1) * 512)
                    nc.vector.tensor_tensor(c7f[:, sl], ch[0][:, sl],
                                            e63q[cch], opc)
                    nc.vector.tensor_tensor(tf[:, sl], tacc[0][:, sl],
                                            c7f[:, sl], opt)
                    nc.tensor.matmul(
                        po,
                        pm[:, 4 * cch : 4 * cch + 4],
                        tf[:, sl],
                        start=(cch == 0),
                        stop=(cch == 3),
                    )

                # Tail on [4,512], partition-parallel.
                z = spool.tile([4, 512], f32, tag="z", name="z")
                nc.vector.tensor_mul(z, po, A4)
                th = spool.tile([4, 512], f32, tag="th", name="th")
                nc.scalar.activation(th, z, AF.Tanh, bias=chs[:, 0:1], scale=0.5)
                ob = spool.tile([4, 512], f32, tag="ob", name="ob")
                nc.vector.tensor_scalar(
                    out=ob, in0=th, scalar1=0.5, scalar2=0.5,
                    op0=ALU.mult, op1=ALU.add,
                )
                nc.sync.dma_start(out=out_d.ap(), in_=ob)

    nc.compile()
    return nc


_NC_CACHE = {}


def _get_nc(p_pos):
    if p_pos not in _NC_CACHE:
        _NC_CACHE[p_pos] = _build_bass(p_pos)
    return _NC_CACHE[p_pos]


def _prep_inputs(x, alphas, xis, yis, intercept):
    x = np.asarray(x, np.float32)
    xis = np.asarray(xis, np.float32)
    alphas = np.asarray(alphas, np.float32)
    yis = np.asarray(yis, np.float32)
    intercept = np.asarray(intercept, np.float32)

    # Sort SVs: positive labels first, so s-tiles are sign-pure except at
    # most one boundary tile.
    pos = np.flatnonzero(yis > 0)
    neg = np.flatnonzero(yis <= 0)
    perm = np.concatenate([pos, neg])
    p_pos = int(pos.size)
    xis_s = xis[perm]
    alphas_s = alphas[perm]

    xT = np.ascontiguousarray(x.T).reshape(2, 128, N).astype(np.float16)
    xisT = np.ascontiguousarray(xis_s.T).reshape(2, 128, S).astype(np.float16)
    xi_sq = np.sum(xis_s * xis_s, axis=1)                  # [S]
    x_sq = np.sum(x * x, axis=1)                           # [N]
    bias = -GAMMA * xi_sq + np.log(np.clip(alphas_s, 1e-30, None))
    biasS = np.ascontiguousarray(bias.reshape(TS, 128).T).astype(np.float32)
    A = np.exp(-GAMMA * x_sq).astype(np.float32)           # [N]
    chalf = (0.5 * intercept[0]) * np.ones((4, 1), np.float32)

    r = p_pos % 128
    sgn = np.ones((128, 1), np.float32)
    if r:
        sgn[r:, 0] = -1.0

    # Final-reduce lhs: block cch is [128,4] with column cch set to the
    # top accumulator's sign convention (sign of the first group's first
    # leaf), other columns zero.
    signs, _ = _leaf_signs(p_pos)
    pm = np.zeros((128, 16), np.float32)
    for cch in range(4):
        pm[:, 4 * cch + cch] = signs[0]

    in_maps = []
    for c in range(NCORES):
        sl = slice(c * NS, (c + 1) * NS)
        in_maps.append(
            {
                "xT": np.ascontiguousarray(xT[:, :, sl]),
                "xisT": xisT,
                "biasS": biasS,
                "A4": np.ascontiguousarray(A[sl]).reshape(4, NS // 4),
                "chalf": chalf,
                "sgn": sgn,
                "pm": pm,
            }
        )
    return in_maps, p_pos


def kernel(x, alphas, xis, yis, intercept, _trace=False):
    from concourse import bass_utils

    in_maps, p_pos = _prep_inputs(x, alphas, xis, yis, intercept)
    nc = _get_nc(p_pos)
    res = bass_utils.run_bass_kernel_spmd(
        nc, in_maps, core_ids=list(range(NCORES)), trace=_trace
    )
    out = np.concatenate(
        [res.results[c]["out"].reshape(1, NS) for c in range(NCORES)], axis=1
    )
    if _trace:
        return out.astype(np.float32), res
    return out.astype(np.float32)


# revision 39
# speedup vs baseline: 1.0088x; 1.0088x over previous
"""Trainium2 Bass kernel for DifferentiableRBFSVMModel forward.

Math (reference):
    dist[n,s] = max(x_sq[n] + xi_sq[s] - 2*cross[n,s], 0)
    K = exp(-g*dist);  res = sigmoid(K @ (alphas*yis) + intercept)   -> [1, N]

Factorization used on device (clamp dropped: dist >= 0 up to fp eps):
    K[n,s]*w[s] = A[n] * sgn_s * exp(2g*cross[n,s] - g*xi_sq[s] + ln(alpha_s))
    res[n] = sigmoid(A[n] * sum_s sgn_s*E[s,n] + c),  A = exp(-g*x_sq)

SVs are host-sorted so all yis=+1 come first: every 128-row s-tile is then
sign-pure except at most one boundary tile (fixed with a per-partition sgn
multiply).  The weighted SV reduction becomes plain +/- adds of E tiles.

Sharding: data-parallel over N across 8 cores, everything else replicated.
Per core (NS = 2048 rows of x, 64 s-tiles):
    mm1 (PE):  psum tiles [128s x 2048n] = xisT_tile^T @ xT  (fp16, fp32 acc)
    exp:       E_t = exp(2g*psum + bias_s) in fp16.  Most stages on ACT
               ([128,2048] instructions); OFFLOAD stages use a custom DVE op
               EXP8_ANT (8 ALU blocks: (c((1+lam*v)^2+b))^8 * C1, C1 =
               c^8*exp(bias) per partition, max rel err ~3e-4) so the two
               engines split the exp wall.
    DVE:       s-reduction off the PE: per-group-of-8 pairwise fp16 trees
               (last group a chain for a short drain), group roots folded
               into ONE fp32 running accumulator TACC (mixed-dtype adds).
    PE tail:   po[4,512] = pm^T @ TACC (4 fp32 matmuls, selector lhs puts
               n-chunk c in partition c; psum pool opened after the mm1
               pool closes), then z = A*po, sigmoid via tanh on [4,512].

DMA: xisT split into 8 column-chunks per d-half; chunks 1+ gated on a marker
read of the previous stage's E tile so their DMA doesn't compete with the
prologue-critical DMAs.
"""

import numpy as np

N, D, S, NCORES = 16384, 256, 8192, 8
NS = N // NCORES          # 2048 rows of x per core
TS = S // 128             # 64 s-tiles
GAMMA = 0.00390625        # 1/256
XCH = 8                   # xisT column chunks per d-half (1024 cols each)
GRP = 8                   # s-tiles per reduction group

# exp(v) ~= (C_EXP*((1+LAM_EXP*v)^2 + B_EXP))^8, minimax on |v|<=0.8
LAM_EXP = 0.124753797
B_EXP = 0.993573014
C8_EXP = 0.004008243562419464
IMM2_EXP = LAM_EXP * 2.0 * GAMMA          # Src0 is raw cross from psum

# Stages whose exp runs on the DVE (custom op) instead of ACT.  Offloads
# only help in CONSECUTIVE PAIRS: with the 2-deep psum ring, a lone skipped
# ACT stage just stalls ACT on the PE; a pair lets ACT and DVE consume
# alternate psum tiles concurrently and the stream goes PE-bound.
OFFLOAD = frozenset()


def _leaf_signs(p_pos):
    """Per-s-tile sign after host sort; boundary tile is sgn-fixed to +1."""
    signs = []
    boundary = -1
    for t in range(TS):
        lo, hi = t * 128, (t + 1) * 128
        if hi <= p_pos:
            signs.append(1)
        elif lo >= p_pos:
            signs.append(-1)
        else:
            signs.append(1)
            boundary = t
    return signs, boundary


_EXP8 = None


def _get_exp8():
    """Register the EXP8_ANT custom DVE op (idempotent per process)."""
    global _EXP8
    if _EXP8 is not None:
        return _EXP8
    import concourse.dve_ops as dops
    from concourse.dve_ops import DveOp
    from concourse.dve_spec import (
        C0, C1, C2, One, Spec, Src0, _has_src1, lower as uop_lower,
    )
    from concourse.dve_uop import DveOpSpec

    w1 = Src0 * C2 + One
    a2 = w1 * w1 + C0
    s1 = a2 * a2
    s2 = s1 * s1
    s3 = s2 * s2
    body = s3 * C1

    def _ref(in0, in1, s0, s1, imm2):
        a2 = (1.0 + imm2 * in0.astype(np.float32)) ** 2 + s0
        return (a2 ** 8) * s1

    spec = Spec(body=body, reference=_ref)
    if "EXP8_ANT" not in dops._SUB_OPCODE_FOR_NAME:
        row = max(dops._SUB_OPCODE_FOR_NAME.values()) + 1
        assert row < 0x20
        uops = uop_lower(spec, ver="v3")
        sha = DveOpSpec(
            name="EXP8_ANT", opcode=row, uops=uops, rd1_en=_has_src1(spec)
        ).sha("v3")
        op = DveOp("EXP8_ANT", spec, subdim=False, uops_sha={"v3": sha})
        dops.OPS.append(op)
        dops._SUB_OPCODE_FOR_NAME["EXP8_ANT"] = row
        dops.CUSTOM_DVE_SPECS["EXP8_ANT"] = spec
        _EXP8 = op
    else:
        _EXP8 = next(o for o in dops.OPS if o.name == "EXP8_ANT")
    return _EXP8


def _build_bass(p_pos):
    import concourse.bacc as bacc
    import concourse.mybir as mybir
    import concourse.tile as tile

    f32 = mybir.dt.float32
    f16 = mybir.dt.float16
    AF = mybir.ActivationFunctionType
    ALU = mybir.AluOpType

    exp8 = _get_exp8() if OFFLOAD else None
    signs, boundary = _leaf_signs(p_pos)

    nc = bacc.Bacc("TRN2", target_bir_lowering=False, debug=False)

    xT_d = nc.dram_tensor("xT", [2, 128, NS], f16, kind="ExternalInput")
    xisT_d = nc.dram_tensor("xisT", [2, 128, S], f16, kind="ExternalInput")
    biasS_d = nc.dram_tensor("biasS", [128, TS], f32, kind="ExternalInput")
    A4_d = nc.dram_tensor("A4", [4, NS // 4], f32, kind="ExternalInput")
    ch_d = nc.dram_tensor("chalf", [4, 1], f32, kind="ExternalInput")
    sgn_d = nc.dram_tensor("sgn", [128, 1], f32, kind="ExternalInput")
    pm_d = nc.dram_tensor("pm", [128, 16], f32, kind="ExternalInput")
    out_d = nc.dram_tensor("out", [4, NS // 4], f32, kind="ExternalOutput")

    cw = S // XCH  # 1024

    with tile.TileContext(nc) as tc:
        with (
            tc.tile_pool(name="big", bufs=1) as big,
            tc.tile_pool(name="epool", bufs=7) as epool,
            tc.tile_pool(name="ppool", bufs=2) as ppool,
            tc.tile_pool(name="qpool", bufs=2) as qpool,
            tc.tile_pool(name="rpool", bufs=2) as rpool,
            tc.tile_pool(name="c7pool", bufs=2) as c7pool,
            tc.tile_pool(name="tpool", bufs=2) as tpool,
            tc.tile_pool(name="spool", bufs=2) as spool,
            tc.tile_pool(name="e63pool", bufs=2) as e63pool,
        ):
            # Prologue-critical DMAs first, interleaved per d-half; small
            # late-use tensors go through the idle GpSimd queue so they
            # don't delay the Sync issue stream.
            xis = {}
            for c in range(XCH):
                for d in range(2):
                    xis[(d, c)] = big.tile(
                        [128, cw], f16, tag=f"xis{d}_{c}", name=f"xis{d}_{c}"
                    )
            xt = []
            for d in range(2):
                t_ = big.tile([128, NS], f16, tag=f"xt{d}", name=f"xt{d}")
                xt.append(t_)
            biasS = big.tile([128, TS], f32, tag="biasS", name="biasS")
            # Column-ordered prologue: stage 0's LEFT-half exp needs only
            # xisT cols 0-127 (weights) and xt cols 0-1023, so those pieces
            # go first, split across the Sync (d=0) and Vector (d=1) issue
            # queues to halve the serial issue time.
            hN = NS // 2
            nc.sync.dma_start(out=xis[(0, 0)][:, 0:128],
                              in_=xisT_d.ap()[0][:, 0:128])
            nc.gpsimd.dma_start(out=xis[(1, 0)][:, 0:128],
                                in_=xisT_d.ap()[1][:, 0:128])
            nc.sync.dma_start(out=xt[0][:, 0:hN], in_=xT_d.ap()[0][:, 0:hN])
            nc.gpsimd.dma_start(out=xt[1][:, 0:hN], in_=xT_d.ap()[1][:, 0:hN])
            nc.sync.dma_start(out=xt[0][:, hN:NS], in_=xT_d.ap()[0][:, hN:NS])
            nc.gpsimd.dma_start(out=xt[1][:, hN:NS], in_=xT_d.ap()[1][:, hN:NS])
            nc.sync.dma_start(out=xis[(0, 0)][:, 128:cw],
                              in_=xisT_d.ap()[0][:, 128:cw])
            nc.gpsimd.dma_start(out=xis[(1, 0)][:, 128:cw],
                                in_=xisT_d.ap()[1][:, 128:cw])
            nc.gpsimd.dma_start(out=biasS, in_=biasS_d.ap())
            A4 = big.tile([4, NS // 4], f32, tag="A4", name="A4")
            nc.gpsimd.dma_start(out=A4, in_=A4_d.ap())
            chs = big.tile([4, 1], f32, tag="chalf", name="chs")
            nc.gpsimd.dma_start(out=chs, in_=ch_d.ap())
            sgn = big.tile([128, 1], f32, tag="sgn", name="sgn")
            nc.gpsimd.dma_start(out=sgn, in_=sgn_d.ap())
            pm = big.tile([128, 16], f32, tag="pm", name="pm")
            nc.gpsimd.dma_start(out=pm, in_=pm_d.ap())

            # Warmup ACTs: attaches the activation-table-load waits here
            # instead of the first pipeline exp.
            wsrc = big.tile([1, 1], f32, tag="wsrc", name="wsrc")
            nc.vector.memset(wsrc, 0.0)
            wdst = big.tile([1, 1], f32, tag="wdst", name="wdst")
            nc.scalar.activation(wdst, wsrc, AF.Tanh)
            nc.scalar.activation(wdst, wsrc, AF.Exp)

            # PE warmup spin: the PE runs at a low P-state for its first ~3us
            # of activity; burn that on dummy matmuls during the prologue DMA
            # wait so stage 0 streams at full clock.
            wmm = big.tile([128, 512], f16, tag="wmm", name="wmm")
            nc.vector.memset(wmm, 0.0)
            with tc.tile_pool(name="psumw", bufs=1, space="PSUM") as psumw:
                pw = psumw.tile([128, 512], f32, tag="pw", name="pw")
                NWARM = 12
                for i in range(NWARM):
                    nc.tensor.matmul(
                        pw, wmm[:, 0:128], wmm,
                        start=(i == 0), stop=(i == NWARM - 1),
                    )
                nc.vector.tensor_copy(wdst, pw[0:1, 0:1])

            gate = big.tile([1, XCH], f32, tag="gate", name="gate")

            # Reduction with compile-time sign bookkeeping: nodes are
            # (tile, sign); combine(a, b) = a.tile +/- b.tile, sign of a.
            pools = {"P": ppool, "Q": qpool, "R": rpool, "C7": c7pool}

            def comb(a, b, tag, name, pool=None, dtype=f16):
                nt = (pool or pools[tag]).tile([128, NS], dtype, tag=tag, name=name)
                op = ALU.add if a[1] == b[1] else ALU.subtract
                nc.vector.tensor_tensor(nt, a[0], b[0], op)
                return (nt, a[1])

            st = {"tacc": None, "slot_l": None, "slot_p": None,
                  "slot_q": None, "pend_root": None, "chain": None}

            def process_tree(leaf, t):
                """Fold stage t's leaf into the reduction state."""
                g, pos = t // GRP, t % GRP
                root = None
                if g == TS // GRP - 1:
                    # Last group: running fp16 chain -> only one combine
                    # after the final leaf (short drain).
                    st["chain"] = (leaf if pos == 0
                                   else comb(st["chain"], leaf, "C7", f"C7_{t}"))
                elif pos % 2 == 0:
                    st["slot_l"] = leaf
                else:
                    p = comb(st["slot_l"], leaf, "P", f"P_{t}")
                    if pos in (1, 5):
                        st["slot_p"] = p
                    elif pos == 3:
                        st["slot_q"] = comb(st["slot_p"], p, "Q", f"Q_{t}")
                    else:  # pos == 7
                        q2 = comb(st["slot_p"], p, "Q", f"Q2_{t}")
                        root = comb(st["slot_q"], q2, "R", f"R_{t}")
                if root is not None:
                    if st["tacc"] is None and st["pend_root"] is None:
                        st["pend_root"] = root
                    else:
                        a = st["tacc"] if st["tacc"] is not None else st["pend_root"]
                        st["pend_root"] = None
                        st["tacc"] = comb(a, root, "T", f"T_{t}", pool=tpool,
                                          dtype=f32)

            last_e = None
            pend = None   # (leaf, t) tree work delayed one stage so an
            #               offloaded stage's DVE exp leads the DVE queue
            #               and releases its psum tile without queue delay.

            with tc.tile_pool(name="psumc", bufs=2, space="PSUM") as psumc:
                for t in range(TS):
                    c, o = t // XCH, (t % XCH) * 128
                    pc = psumc.tile([128, NS], f32, tag="pc", name=f"pc_{t}")
                    # Stage 0 runs q-major so its left-half psum completes
                    # from the first DMA pieces; later stages run d-major
                    # (2 weight loads per stage instead of 8).
                    dq = ([(d, q) for q in range(4) for d in range(2)]
                          if t == 0 else
                          [(d, q) for d in range(2) for q in range(4)])
                    for d, q in dq:
                        nc.tensor.matmul(
                            pc[:, q * 512 : (q + 1) * 512],
                            xis[(d, c)][:, o : o + 128],
                            xt[d][:, q * 512 : (q + 1) * 512],
                            start=(d == 0),
                            stop=(d == 1),
                        )
                    if t == TS - 1:
                        eh = []
                        for hf in range(2):
                            sl = slice(hf * (NS // 2), (hf + 1) * (NS // 2))
                            et = e63pool.tile([128, NS // 2], f16, tag="E63",
                                              name=f"E63_{hf}")
                            nc.scalar.activation(
                                et, pc[:, sl], AF.Exp,
                                bias=biasS[:, t : t + 1], scale=2.0 * GAMMA,
                            )
                            if t == boundary:
                                nc.vector.tensor_scalar(
                                    out=et, in0=et, scalar1=sgn[:, 0:1],
                                    scalar2=None, op0=ALU.mult,
                                )
                            eh.append(et)
                        pend = (eh, signs[t])
                        continue
                    e = epool.tile([128, NS], f16, tag="E", name=f"E_{t}")
                    if t in OFFLOAD:
                        nc.vector._custom_dve(
                            exp8, out=e, in0=pc, s0=B_EXP,
                            s1=expb[:, t : t + 1], imm2=IMM2_EXP,
                        )
                    else:
                        nc.scalar.activation(
                            e, pc, AF.Exp, bias=biasS[:, t : t + 1],
                            scale=2.0 * GAMMA,
                        )
                    if t == boundary:
                        ef = epool.tile([128, NS], f16, tag="E", name=f"Ef_{t}")
                        nc.vector.tensor_scalar(
                            out=ef, in0=e, scalar1=sgn[:, 0:1], scalar2=None,
                            op0=ALU.mult,
                        )
                        e = ef

                    # Gated xisT chunk DMAs: marker waits on the previous
                    # stage's E (SBUF; no extra psum reader), the DMA
                    # WAW-waits on the marker.
                    if t % 4 == 0 and t // 4 + 1 < XCH:
                        cn = t // 4 + 1
                        msrc = last_e if last_e is not None else xt[1]
                        nc.vector.tensor_copy(gate[0:1, cn : cn + 1], msrc[0:1, 0:1])
                        for d in range(2):
                            nc.vector.tensor_copy(
                                xis[(d, cn)][0:1, 0:1], gate[0:1, cn : cn + 1]
                            )
                            nc.sync.dma_start(
                                out=xis[(d, cn)],
                                in_=xisT_d.ap()[d][:, cn * cw : (cn + 1) * cw],
                            )
                    last_e = e

                    process_tree((e, signs[t]), t)

            # Drain, column-split in quarters so the last chain combine, the
            # last fp32 fold and the final matmuls pipeline: the final leaf
            # joins the chain per quarter, folds into TACC per quarter, and
            # each quarter's matmul starts while the next is still on DVE.
            eh, lsign = pend
            ch = st["chain"]
            opc = ALU.add if ch[1] == lsign else ALU.subtract
            c7f = c7pool.tile([128, NS], f16, tag="C7", name="C7f")
            tacc = st["tacc"]
            opt = ALU.add if tacc[1] == ch[1] else ALU.subtract
            tf = tpool.tile([128, NS], f32, tag="T", name="Tf")

            # Final partition reduction into one [4,512] psum tile at
            # partitions 0-3: 4 fp32 matmuls; pm block cch is +-tacc.sign in
            # column cch and 0 elsewhere, so n-chunk cch lands in row cch.
            with tc.tile_pool(name="psumo", bufs=1, space="PSUM") as psumo:
                po = psumo.tile([4, 512], f32, tag="po", name="po")
                for cch in range(4):
                    sl = slice(cch * 512, (cch + 1) * 512)
                    esrc = eh[cch // 2]
                    ecol = (cch % 2) * 512
                    nc.vector.tensor_tensor(c7f[:, sl], ch[0][:, sl],
                                            esrc[:, ecol : ecol + 512], opc)
                    nc.vector.tensor_tensor(tf[:, sl], tacc[0][:, sl],
                                            c7f[:, sl], opt)
                    nc.tensor.matmul(
                        po,
                        pm[:, 4 * cch : 4 * cch + 4],
                        tf[:, sl],
                        start=(cch == 0),
                        stop=(cch == 3),
                    )

                # Tail on [4,512], partition-parallel.
                z = spool.tile([4, 512], f32, tag="z", name="z")
                nc.vector.tensor_mul(z, po, A4)
                th = spool.tile([4, 512], f32, tag="th", name="th")
                nc.scalar.activation(th, z, AF.Tanh, bias=chs[:, 0:1], scale=0.5)
                ob = spool.tile([4, 512], f32, tag="ob", name="ob")
                nc.vector.tensor_scalar(
                    out=ob, in0=th, scalar1=0.5, scalar2=0.5,
                    op0=ALU.mult, op1=ALU.add,
                )
                nc.gpsimd.dma_start(out=out_d.ap(), in_=ob)

    nc.compile()
    return nc


_NC_CACHE = {}


def _get_nc(p_pos):
    if p_pos not in _NC_CACHE:
        _NC_CACHE[p_pos] = _build_bass(p_pos)
    return _NC_CACHE[p_pos]


def _prep_inputs(x, alphas, xis, yis, intercept):
    x = np.asarray(x, np.float32)
    xis = np.asarray(xis, np.float32)
    alphas = np.asarray(alphas, np.float32)
    yis = np.asarray(yis, np.float32)
    intercept = np.asarray(intercept, np.float32)

    # Sort SVs: positive labels first, so s-tiles are sign-pure except at
    # most one boundary tile.
    pos = np.flatnonzero(yis > 0)
    neg = np.flatnonzero(yis <= 0)
    perm = np.concatenate([pos, neg])
    p_pos = int(pos.size)
    xis_s = xis[perm]
    alphas_s = alphas[perm]

    xT = np.ascontiguousarray(x.T).reshape(2, 128, N).astype(np.float16)
    xisT = np.ascontiguousarray(xis_s.T).reshape(2, 128, S).astype(np.float16)
    xi_sq = np.sum(xis_s * xis_s, axis=1)                  # [S]
    x_sq = np.sum(x * x, axis=1)                           # [N]
    bias = -GAMMA * xi_sq + np.log(np.clip(alphas_s, 1e-30, None))
    biasS = np.ascontiguousarray(bias.reshape(TS, 128).T).astype(np.float32)
    A = np.exp(-GAMMA * x_sq).astype(np.float32)           # [N]
    chalf = (0.5 * intercept[0]) * np.ones((4, 1), np.float32)

    r = p_pos % 128
    sgn = np.ones((128, 1), np.float32)
    if r:
        sgn[r:, 0] = -1.0

    # Final-reduce lhs: block cch is [128,4] with column cch set to the
    # top accumulator's sign convention (sign of the first group's first
    # leaf), other columns zero.
    signs, _ = _leaf_signs(p_pos)
    pm = np.zeros((128, 16), np.float32)
    for cch in range(4):
        pm[:, 4 * cch + cch] = signs[0]

    in_maps = []
    for c in range(NCORES):
        sl = slice(c * NS, (c + 1) * NS)
        in_maps.append(
            {
                "xT": np.ascontiguousarray(xT[:, :, sl]),
                "xisT": xisT,
                "biasS": biasS,
                "A4": np.ascontiguousarray(A[sl]).reshape(4, NS // 4),
                "chalf": chalf,
                "sgn": sgn,
                "pm": pm,
            }
        )
    return in_maps, p_pos


def kernel(x, alphas, xis, yis, intercept, _trace=False):
    from concourse import bass_utils

    in_maps, p_pos = _prep_inputs(x, alphas, xis, yis, intercept)
    nc = _get_nc(p_pos)
    res = bass_utils.run_bass_kernel_spmd(
        nc, in_maps, core_ids=list(range(NCORES)), trace=_trace
    )
    out = np.concatenate(
        [res.results[c]["out"].reshape(1, NS) for c in range(NCORES)], axis=1
    )
    if _trace:
        return out.astype(np.float32), res
    return out.astype(np.float32)
